# revision 3
# baseline (speedup 1.0000x reference)
"""Causal attention head (S=8192, De=dim=256) on 8 trn2 NeuronCores — fp8 version.

Math: score = (x Wq^T Wk x^T)/16, causal-masked softmax, out = attn @ (Wv x^T)^T.

Design (per core, rows c::8 stride-interleaved; kernel SPMD-identical, all
per-core variation in inputs):
  - Fold K: M8 = fp8(32*Wq^T@Wk) host-side; QmT8 = fp8(M8^T @ xq8^T) on device;
    st_psum = 512*score via ONE fp8 DoubleRow matmul per 128-col tile
    (contraction 256 = 2x128 groups packed in the free dim).
  - Fold V: B = P @ x accumulated in PSUM (DoubleRow fp8 on tile PAIRS);
    epilogue out-chunk = (pd*xq + B^T-read) @ Wv^T in bf16. Rowsums ride the
    B banks via pt-stationary matmuls with a 2-col ones moving operand.
  - Diagonal OUT of the fp8 fabric (pattern-zeroed): d = 512*s_qq via
    prod16 = qm_psum*xq16 + ones-matmul; pd = exp(d/512 - 1) on ACT; folded
    into rowsum (DVE add) and output (pd-scaled xq stationary into epi psum).
  - pt = exp(score - 1): off-diag scores in [-5.7, 5.9] land in fp8 normal
    range (max e^4.9=134 < 240, no overflow/underflow).
  - Causality: noncausal computed cells get -7680 (=-15 in score units) added
    pre-exp via one identity-stationary matmul per band tile (I8=32I x -240
    patterns); exp then gives 0 in fp8.
  - exp split across ACT (true exp, fp8 out) / DVE / GPSIMD (Schraudolph bits
    = st*a + b -> uint8, saturating at 0 on underflow; bitcast to fp8).
  - Rows with local u<64 (n_keys<=512) recomputed fully in bf16 (patch).
"""

import sys

sys.path.insert(0, "/opt/trn_rl_repo")

import math
from collections import deque
from contextlib import ExitStack

import ml_dtypes
import numpy as np

import concourse.bass as bass
import concourse.mybir as mybir
import concourse.tile as tile
from concourse import bacc
from concourse.bass_utils import run_bass_kernel_spmd

BF16 = mybir.dt.bfloat16
F32 = mybir.dt.float32
FP8 = mybir.dt.float8e4
U8 = mybir.dt.uint8
NPBF16 = ml_dtypes.bfloat16
NPFP8 = ml_dtypes.float8_e4m3
DR = mybir.MatmulPerfMode.DoubleRow
EXP = mybir.ActivationFunctionType.Exp
CPY = mybir.ActivationFunctionType.Copy
MUL = mybir.AluOpType.mult
ADD = mybir.AluOpType.add

S, DIM, DE = 8192, 256, 256
NCORES = 8
RPC = S // NCORES           # 1024 local rows per core
NRC = 4                     # row chunks of 256
G = 4                       # col tiles per exp group
MSCALE = 32.0               # M8 = fp8(32 * Wq^T Wk)  ->  st = 512*score
STS = 512.0
C_OFF = 1.0                 # global softmax shift: pt = exp(score - 1)
A_SCH = 8 * math.log2(math.e) / STS
B_SCH = 56.0 - 8 * math.log2(math.e) * C_OFF - 0.3
PATCH = 64
OFFS = [0, 16, 48, 96]      # even-group pattern offsets (widths 16,32,48,64)
OFFS_O = [160, 240, 336, 448]  # odd-group pattern offsets (widths 80,96,112,128)

_cached = {}


class Sched:
    """Greedy engine load balancer for elementwise work (load in us)."""

    def __init__(self, nc):
        self.nc = nc
        self.bias = None
        self.load = {"act": 0.0, "dve": 0.0, "pool": 0.0}
        self.rate = {"act": 0.833e-3, "dve": 1.0417e-3, "pool": 1.389e-3}
        self.ovh = {"act": 0.24, "dve": 0.20, "pool": 0.26}

    def pick(self, nelem, allowed=("act", "dve", "pool")):
        best, cost = None, None
        for e in allowed:
            c = self.load[e] + nelem * self.rate[e] + self.ovh[e]
            if cost is None or c < cost:
                best, cost = e, c
        self.load[best] = cost
        return best

    def add(self, eng, nelem):
        self.load[eng] += nelem * self.rate[eng] + self.ovh[eng]

    def exp(self, out_ap, in_ap, nelem):
        e = self.pick(nelem, ("act", "dve"))
        if e == "act":
            self.nc.scalar.activation(out_ap, in_ap, EXP, scale=1.0 / STS,
                                      bias=self.bias)
        elif e == "dve":
            self.nc.vector.tensor_scalar(
                out_ap.bitcast(U8), in_ap, A_SCH, B_SCH, MUL, ADD)
        else:
            self.nc.gpsimd.tensor_scalar(
                out_ap.bitcast(U8), in_ap, A_SCH, B_SCH, MUL, ADD)

    def copy(self, out_ap, in_ap, nelem, allowed=("act", "dve")):
        e = self.pick(nelem, allowed)
        if e == "act":
            self.nc.scalar.copy(out_ap, in_ap)
        elif e == "dve":
            self.nc.vector.tensor_copy(out_ap, in_ap)
        else:
            self.nc.gpsimd.tensor_copy(out_ap, in_ap)

    def mul2(self, out_ap, a_ap, b_ap, nelem):
        self.pick(nelem, ("dve",))
        self.nc.vector.tensor_mul(out_ap, a_ap, b_ap)


def _build_nc():
    nc = bacc.Bacc("TRN2", target_bir_lowering=False, debug=False,
                   num_devices=NCORES)
    # Packed DRAM inputs (fewer DMA instructions; HWDGE is serialized):
    #   cb8:  m8[512] | i8[128] | patt8[160] | xq8[2048]        fp8 [128, 2848]
    #   cb16a: xq16[2048] | wvt[512]                            bf16 [128, 2560]
    #   cbp:  xtp[1024] | xrp[1024] | pattp[160] | i16[128]     bf16 [128, 2336]
    cb8d = nc.dram_tensor("cb8", [128, 3264], FP8, kind="ExternalInput")
    cb16ad = nc.dram_tensor("cb16a", [128, 2560], BF16, kind="ExternalInput")
    cbpd = nc.dram_tensor("cbp", [128, 2848], BF16, kind="ExternalInput")
    i32d = nc.dram_tensor("i32", [128, 128], F32, kind="ExternalInput")
    xt8d = nc.dram_tensor("xt8", [2, 128, S], FP8, kind="ExternalInput")
    xr8d = nc.dram_tensor("xr8", [128, S * 2], FP8, kind="ExternalInput")
    outd = nc.dram_tensor("out", [RPC, DE], BF16, kind="ExternalOutput")

    with tile.TileContext(nc) as tc, ExitStack() as ctx:
        const = ctx.enter_context(tc.tile_pool(name="const", bufs=1))
        ps_st = ctx.enter_context(tc.tile_pool(name="ps_st", bufs=3, space="PSUM"))
        ps_b = ctx.enter_context(tc.tile_pool(name="ps_b", bufs=1, space="PSUM"))
        pt_pool = ctx.enter_context(tc.tile_pool(name="pt", bufs=7))
        sb_pool = ctx.enter_context(tc.tile_pool(name="sb", bufs=2))
        os_pool = ctx.enter_context(tc.tile_pool(name="os", bufs=3))
        _emit(nc, const, ps_st, ps_b, pt_pool, sb_pool,
              os_pool, cb8d, cb16ad, cbpd, i32d, xt8d, xr8d, outd)

    nc.compile()
    return nc


def _emit(nc, const, ps_st, ps_b, pt_pool, sb_pool,
          os_pool, cb8d, cb16ad, cbpd, i32d, xt8d, xr8d, outd):
    sch = Sched(nc)

    # ---- SBUF constants / staged inputs ----
    cb8 = const.tile([128, 3264], FP8, tag="cb8")
    cb16a = const.tile([128, 2560], BF16, tag="cb16a")
    cbp = const.tile([128, 2848], BF16, tag="cbp")
    m8 = cb8[:, 0:512]
    i8 = cb8[:, 512:640]
    patt8 = cb8[:, 640:1216]
    xq8 = cb8[:, 1216:3264]
    xq16 = cb16a[:, 0:2048]
    wvt = cb16a[:, 2048:2560]
    xtp = cbp[:, 0:1024]
    xrp = cbp[:, 1024:2048]
    pattp = cbp[:, 2048:2208]
    i16 = cbp[:, 2208:2336]
    m16 = cbp[:, 2336:2848]
    xt8 = const.tile([128, 2 * S], FP8, tag="xt8")
    xr8 = const.tile([128, 2 * S], FP8, tag="xr8")
    i32 = const.tile([128, 128], F32, tag="i32")
    qmt8 = const.tile([128, 2 * RPC], FP8, tag="qmt8")
    qmtp = const.tile([128, 2 * PATCH], BF16, tag="qmtp")
    prod16 = const.tile([128, 2 * RPC], BF16, tag="prod16")
    ones16 = const.tile([128, 1], BF16, tag="ones16")
    ones8 = const.tile([128, 2], FP8, tag="ones8")
    bias_m1 = const.tile([128, 1], F32, tag="bias_m1")
    pd_sb = const.tile([128, 2 * NRC], F32, tag="pd_sb")
    pdt_sb = [const.tile([1, 128], F32, tag=f"pdt{h}", name=f"pdt{h}")
              for h in range(2)]
    pdb_sb = [const.tile([128, 128], F32, tag=f"pdb{h}", name=f"pdb{h}")
              for h in range(2)]
    xqpd = const.tile([128, 256], BF16, tag="xqpd")
    rinv_sb = const.tile([128, 2], F32, tag="rinv_sb")
    rstot = const.tile([128, 2], F32, tag="rstot")

    nc.gpsimd.memset(ones16[:], 1.0)
    nc.gpsimd.memset(ones8[:], 1.0)
    nc.gpsimd.memset(bias_m1[:], -C_OFF)
    sch.bias = bias_m1[:]

    # ---- DMAs (single SP queue; HWDGE+DMA engines serialize globally,
    # so order = consumption order, minimal instruction count) ----
    nc.sync.dma_start(cb8[:], cb8d[:, :])
    for h in range(2):
        nc.sync.dma_start(xt8[:, h * S:h * S + 512], xt8d[h, :, 0:512])
    for h in range(2):
        nc.sync.dma_start(xt8[:, h * S + 512:h * S + 2048],
                          xt8d[h, :, 512:2048])
    nc.sync.dma_start(xr8[:, 0:4096], xr8d[:, 0:4096])
    nc.sync.dma_start(cb16a[:], cb16ad[:, :])
    nc.sync.dma_start(i32[:], i32d[:, :])
    for o in range(2048, S, 2048):
        for h in range(2):
            nc.sync.dma_start(xt8[:, h * S + o:h * S + o + 2048],
                              xt8d[h, :, o:o + 2048])
        nc.sync.dma_start(xr8[:, 2 * o:2 * o + 4096],
                          xr8d[:, 2 * o:2 * o + 4096])
        if o == 2048:
            nc.sync.dma_start(cbp[:], cbpd[:, :])

    # views
    xt8v = xt8[:].rearrange("p (h s) -> p h s", h=2)      # [128, 2, 8192]
    m8v = m8.rearrange("p (h e) -> p h e", h=2)           # [128, 2, 256]
    qmt8v = qmt8[:].rearrange("p (h q) -> p h q", h=2)    # [128, 2, 1024]
    xq8v = xq8.rearrange("p (h q) -> p h q", h=2)         # [128, 2, 1024]
    xr8v = xr8[:].rearrange("p (c e) -> p c e", e=256)    # [128, 64, 256]
    ones8v = ones8[:].rearrange("p (t o) -> p t o", t=2)  # [128, 2, 1]

    # PSUM banks: ps_st 3x2 + ps_b 2 = 8; qm/epi tiles rotate in ps_st
    b_ps = [ps_b.tile([128, 512], F32, tag=f"b{eh}", name=f"b{eh}")
            for eh in range(2)]

    def emit_qm(r):
        q0 = r * 256
        qm_ps = ps_st.tile([128, G * 256], F32, tag="st", name="qm")
        for eh in range(2):
            nc.tensor.matmul(
                qm_ps[:, eh * 256:(eh + 1) * 256],
                m8v[:, :, eh * 128:(eh + 1) * 128],
                xq8v[:, :, q0:q0 + 256],
                start=True, stop=True, perf_mode=DR)
        for eh in range(2):
            sch.copy(qmt8[:, eh * RPC + q0:eh * RPC + q0 + 256],
                     qm_ps[:, eh * 256:(eh + 1) * 256], 256)
        for eh in range(2):
            nc.gpsimd.tensor_mul(
                prod16[:, eh * RPC + q0:eh * RPC + q0 + 256],
                qmt8[:, eh * RPC + q0:eh * RPC + q0 + 256],
                xq8[:, eh * RPC + q0:eh * RPC + q0 + 256])
            sch.add("pool", 256)
        for h in range(2):
            for eh in range(2):
                nc.tensor.matmul(
                    b_ps[h][:, 257:258],
                    prod16[:, eh * RPC + q0 + h * 128:
                           eh * RPC + q0 + h * 128 + 128],
                    ones16[:, 0:1],
                    start=(r == 0 and eh == 0), stop=False,
                    skip_group_check=True)
        # pd chain: pd = exp(d/512 - 1); pdT via PE transpose; broadcast to
        # pdb via SWDGE (stride-0 source) on the otherwise-idle Pool engine.
        for h in range(2):
            nc.scalar.activation(pd_sb[:, 2 * r + h:2 * r + h + 1],
                                 b_ps[h][:, 257:258], EXP, scale=1.0 / STS,
                                 bias=bias_m1[:])
            sch.add("act", 4)
        for h in range(2):
            nc.tensor.matmul(
                qm_ps[0:1, 512 + h * 128:640 + h * 128],
                pd_sb[:, 2 * r + h:2 * r + h + 1], i32[:],
                start=True, stop=True, is_transpose=True,
                skip_group_check=True)
            nc.vector.tensor_copy(pdt_sb[h][:],
                                  qm_ps[0:1, 512 + h * 128:640 + h * 128])
            sch.add("dve", 64)
            nc.gpsimd.partition_broadcast(pdb_sb[h][:], pdt_sb[h][:])

    def emit_patch():
        # QmTp = (xq16 @ M16)^T in bf16 (clean of fp8 error) for patch rows
        qmp_ps = ps_st.tile([128, G * 256], F32, tag="st", name="qmp")
        for e2 in range(2):
            for i in range(2):
                nc.tensor.matmul(
                    qmp_ps[:, e2 * 64:(e2 + 1) * 64],
                    m16[:, i * 256 + e2 * 128:i * 256 + e2 * 128 + 128],
                    xq16[:, i * RPC:i * RPC + PATCH],
                    start=(i == 0), stop=(i == 1), skip_group_check=True)
        for e2 in range(2):
            nc.scalar.copy(qmtp[:, e2 * PATCH:(e2 + 1) * PATCH],
                           qmp_ps[:, e2 * 64:(e2 + 1) * 64])
            sch.add("act", PATCH)
        st_p = ps_st.tile([128, G * 256], F32, tag="st")
        for j in range(4):
            for eh in range(2):
                nc.tensor.matmul(
                    st_p[:, j * 64:(j + 1) * 64],
                    xtp[:, eh * 8 * PATCH + j * 128:
                        eh * 8 * PATCH + (j + 1) * 128],
                    qmtp[:, eh * PATCH:(eh + 1) * PATCH],
                    start=(eh == 0), stop=(eh == 1))
            wdt = 16 * (j + 1)
            nc.tensor.matmul(
                st_p[:, j * 64:j * 64 + wdt],
                i16, pattp[:, OFFS[j]:OFFS[j] + wdt],
                start=False, stop=False, skip_group_check=True)
        ptp = pt_pool.tile([128, G * 256], BF16, tag="ptp")
        nc.scalar.activation(ptp[:, 0:256], st_p[:, 0:256], EXP,
                             scale=1.0 / 16)
        sch.add("act", 256)
        for j in range(4):
            for eh in range(2):
                nc.tensor.matmul(
                    b_ps[eh][:, 0:PATCH],
                    xrp[:, j * 256 + eh * 128:j * 256 + eh * 128 + 128],
                    ptp[:, j * 64:(j + 1) * 64],
                    start=(j == 0), stop=(j == 3))
            nc.tensor.matmul(
                b_ps[0][0:PATCH, 256:257],
                ptp[:, j * 64:(j + 1) * 64], ones16[:, 0:1],
                start=False, stop=False, skip_group_check=True)
        bp_sb = sb_pool.tile([128, 512], BF16, tag="bsb")
        for eh in range(2):
            sch.copy(bp_sb[:, eh * PATCH:(eh + 1) * PATCH],
                     b_ps[eh][:, 0:PATCH], PATCH, allowed=("act", "dve"))
        epip = ps_st.tile([128, G * 256], F32, tag="st", name="epip")
        for eh in range(2):
            nc.tensor.matmul(
                epip[0:PATCH, 0:256],
                bp_sb[:, eh * PATCH:(eh + 1) * PATCH],
                wvt[:, eh * 256:(eh + 1) * 256],
                start=(eh == 0), stop=(eh == 1), skip_group_check=True)
        rp = rinv_sb[0:PATCH, 0:1]
        nc.vector.reciprocal(rp, b_ps[0][0:PATCH, 256:257])
        sch.add("dve", 8)
        osb = os_pool.tile([128, 256], BF16, tag="osb")
        nc.vector.tensor_scalar_mul(osb[0:PATCH, :], epip[0:PATCH, 0:256], rp)
        sch.add("dve", 256)
        nc.sync.dma_start(outd[0:PATCH, :], osb[0:PATCH, :])

    def emit_pv(pt, g, r, u0g, w, last):
        # tiles live at 256-col stride in pt regardless of w (bank alignment)
        ptv = pt[:].rearrange("p (t x) -> p t x", x=256)[:, :, 0:w]
        for p in range(2):
            j0 = G * g + 2 * p
            for eh in range(2):
                nc.tensor.matmul(
                    b_ps[eh][:, u0g:256],
                    xr8v[:, j0:j0 + 2, eh * 128:(eh + 1) * 128],
                    ptv[:, 2 * p:2 * p + 2, :],
                    start=(g == 0 and p == 0), stop=(last and p == 1),
                    perf_mode=DR, skip_group_check=True)
            for h in range(2):
                qs = max(u0g, h * 128)
                qe = (h + 1) * 128
                if qs >= qe:
                    continue
                nc.tensor.matmul(
                    b_ps[h][qs - h * 128:qe - h * 128, 256:257],
                    ptv[:, 2 * p:2 * p + 2, qs - u0g:qe - u0g],
                    ones8v,
                    start=False, stop=False, perf_mode=DR,
                    skip_group_check=True)

    def emit_chunk(r):
        ngroups = 4 * (r + 1)
        pending = deque()
        for g in range(ngroups):
            band_pos = g - 4 * r
            u0g = 128 * (band_pos // 2) if band_pos >= 0 else 0
            w = 256 - u0g
            st = ps_st.tile([128, G * 256], F32, tag="st")
            for t in range(G):
                j = G * g + t
                nc.tensor.matmul(
                    st[:, t * 256:t * 256 + w],
                    xt8v[:, :, j * 128:(j + 1) * 128],
                    qmt8v[:, :, r * 256 + u0g:(r + 1) * 256],
                    start=True, stop=True, perf_mode=DR)
                if band_pos >= 0:
                    odd = band_pos % 2
                    wdt = 16 * (t + 1) + 64 * odd
                    off = (OFFS_O if odd else OFFS)[t]
                    nc.tensor.matmul(
                        st[:, t * 256:t * 256 + wdt],
                        i8, patt8[:, off:off + wdt],
                        start=False, stop=False, skip_group_check=True)
            if g == 1 and r < NRC - 1:
                emit_qm(r + 1)
            if len(pending) >= 4:
                emit_pv(*pending.popleft())
            pt = pt_pool.tile([128, G * 256], FP8, tag="pt")
            stv = st[:].rearrange("p (t x) -> p t x", x=256)[:, :, 0:w]
            ptv = pt[:].rearrange("p (t x) -> p t x", x=256)[:, :, 0:w]
            sch.exp(ptv, stv, 4 * w)
            pending.append((pt, g, r, u0g, w, g == ngroups - 1))
        while pending:
            emit_pv(*pending.popleft())

    def emit_epilogue(r):
        epi = ps_st.tile([128, G * 256], F32, tag="st", name="epi")
        for h in range(2):
            nc.vector.tensor_add(
                rstot[:, h:h + 1], b_ps[h][:, 256:257],
                pd_sb[:, 2 * r + h:2 * r + h + 1])
            nc.vector.reciprocal(rinv_sb[:, h:h + 1], rstot[:, h:h + 1])
            sch.add("dve", 16)
            for eh in range(2):
                nc.gpsimd.tensor_mul(
                    xqpd[:, eh * 128:(eh + 1) * 128],
                    xq16[:, eh * RPC + r * 256 + h * 128:
                         eh * RPC + r * 256 + h * 128 + 128],
                    pdb_sb[h][:])
                sch.add("pool", 128)
            b_sb = sb_pool.tile([128, 512], BF16, tag="bsb")
            for eh in range(2):
                sch.copy(b_sb[:, eh * 256:(eh + 1) * 256],
                         b_ps[eh][:, 0:256], 256, allowed=("dve",))
            er = epi[:, h * 256:(h + 1) * 256]
            for eh in range(2):
                nc.tensor.matmul(
                    er, xqpd[:, eh * 128:(eh + 1) * 128],
                    wvt[:, eh * 256:(eh + 1) * 256],
                    start=(eh == 0), stop=False, skip_group_check=True)
            for eh in range(2):
                nc.tensor.matmul(
                    er, b_sb[:, eh * 256 + h * 128:eh * 256 + h * 128 + 128],
                    wvt[:, eh * 256:(eh + 1) * 256],
                    start=False, stop=(eh == 1), skip_group_check=True)
            osb = os_pool.tile([128, 256], BF16, tag="osb")
            rows0 = r * 256 + h * 128
            if r == 0 and h == 0:
                nc.vector.tensor_scalar_mul(
                    osb[PATCH:128, :], er[PATCH:128, :],
                    rinv_sb[PATCH:128, h:h + 1])
                sch.add("dve", 256)
                nc.sync.dma_start(outd[rows0 + PATCH:rows0 + 128, :],
                                  osb[PATCH:128, :])
            else:
                sch.pick(256, ("dve",))
                nc.vector.tensor_scalar_mul(osb[:], er,
                                            rinv_sb[:, h:h + 1])
                nc.sync.dma_start(outd[rows0:rows0 + 128, :], osb[:])

    # ---------- emission order: qm(r+1) emitted mid-chunk(r); patch after
    # epilogue(1) so its consts (cbp DMA) have arrived ----------
    emit_qm(0)
    for r in range(NRC):
        emit_chunk(r)
        emit_epilogue(r)
        if r == 1:
            emit_patch()


def _host_inputs(x, Wq, Wk, Wv):
    def two_half(a, width):            # [256, w] -> [128, 2*w] (half-major)
        return np.ascontiguousarray(
            a.reshape(2, 128, width).transpose(1, 0, 2).reshape(128, 2 * width))

    xT = np.ascontiguousarray(x.T)
    m8 = two_half((MSCALE * (Wq.T @ Wk)).astype(np.float32), 256)
    i8 = 32.0 * np.eye(128, dtype=np.float32)
    i16 = 16.0 * np.eye(128, dtype=np.float32)
    i32 = np.eye(128, dtype=np.float32)
    wvt = two_half(np.ascontiguousarray(Wv.T), 256)
    xtp = two_half(xT[:, :8 * PATCH], 8 * PATCH)
    xrp = np.ascontiguousarray(
        x[:4 * 128].reshape(4, 128, 256).transpose(1, 0, 2).reshape(128, 1024))
    xt8 = np.ascontiguousarray(xT.astype(NPFP8).reshape(2, 128, S))
    xr8 = np.ascontiguousarray(
        x.reshape(64, 128, 256).transpose(1, 0, 2).reshape(128, 2 * S)
    ).astype(NPFP8)
    k_idx = np.arange(128)[:, None]
    in_maps = []
    for c in range(NCORES):
        xq = np.ascontiguousarray(x[c::NCORES].T)    # [256, 1024]
        xq16 = two_half(xq, RPC)
        patt = np.zeros((128, 576), np.float32)      # main: kill k >= 8w+c
        pattp = np.zeros((128, 160), np.float32)     # patch: kill k > 8w+c
        for t in range(4):
            wdt = 16 * (t + 1)
            w_idx = np.arange(wdt)[None, :]
            kk = 128 * t + k_idx
            patt[:, OFFS[t]:OFFS[t] + wdt] = np.where(
                kk >= (8 * w_idx + c), -240.0, 0.0)
            pattp[:, OFFS[t]:OFFS[t] + wdt] = np.where(
                kk > (8 * w_idx + c), -240.0, 0.0)
            # odd band groups start 64 rows before the diagonal block:
            # kill iff 128t + 512 + k >= 8w + c, width 16(t+1)+64
            wdt_o = 16 * (t + 1) + 64
            w_idx = np.arange(wdt_o)[None, :]
            patt[:, OFFS_O[t]:OFFS_O[t] + wdt_o] = np.where(
                (128 * t + 512 + k_idx) >= (8 * w_idx + c), -240.0, 0.0)
        cb8 = np.concatenate(
            [m8, i8, patt, xq16], axis=1).astype(NPFP8)
        cb16a = np.concatenate([xq16, wvt], axis=1).astype(NPBF16)
        m16 = two_half((Wq.T @ Wk).astype(np.float32), 256)
        cbp = np.concatenate([xtp, xrp, pattp, i16, m16],
                             axis=1).astype(NPBF16)
        in_maps.append({
            "cb8": np.ascontiguousarray(cb8),
            "cb16a": np.ascontiguousarray(cb16a),
            "cbp": np.ascontiguousarray(cbp),
            "i32": i32, "xt8": xt8, "xr8": xr8,
        })
    return in_maps


def _host_inputs_old(x, Wq, Wk, Wv):
    M8 = (MSCALE * (Wq.T @ Wk)).astype(NPFP8)
    m8 = np.ascontiguousarray(
        M8.reshape(2, 128, 256).transpose(1, 0, 2).reshape(128, 512))
    xT = np.ascontiguousarray(x.T)
    xt8 = np.ascontiguousarray(xT.astype(NPFP8).reshape(2, 128, S))
    xr8 = np.ascontiguousarray(
        x.reshape(64, 128, 256).transpose(1, 0, 2).reshape(128, 2 * S)
    ).astype(NPFP8)
    xtp = np.ascontiguousarray(
        xT[:, :8 * PATCH].astype(NPBF16).reshape(2, 128, 8 * PATCH))
    xrp = np.ascontiguousarray(
        x[:4 * 128].reshape(4, 128, 256).transpose(1, 0, 2).reshape(128, 1024)
    ).astype(NPBF16)
    wvt = np.ascontiguousarray(Wv.T).astype(NPBF16).reshape(2, 128, 256)
    i8 = (32.0 * np.eye(128, dtype=np.float32)).astype(NPFP8)
    i16 = (16.0 * np.eye(128, dtype=np.float32)).astype(NPBF16)
    i32 = np.eye(128, dtype=np.float32)
    k_idx = np.arange(128)[:, None]
    in_maps = []
    for c in range(NCORES):
        xq = np.ascontiguousarray(x[c::NCORES].T)
        xq16 = xq.astype(NPBF16).reshape(2, 128, RPC)
        xq8c = xq.astype(NPFP8).reshape(2, 128, RPC)
        patt = np.zeros((128, 160), np.float32)    # main: kill k >= 8w+c (diag out)
        pattp = np.zeros((128, 160), np.float32)   # patch: kill k > 8w+c (diag in)
        for t in range(4):
            wdt = 16 * (t + 1)
            w_idx = np.arange(wdt)[None, :]
            kk = 128 * t + k_idx
            patt[:, OFFS[t]:OFFS[t] + wdt] = np.where(
                kk >= (8 * w_idx + c), -240.0, 0.0)
            pattp[:, OFFS[t]:OFFS[t] + wdt] = np.where(
                kk > (8 * w_idx + c), -240.0, 0.0)
        in_maps.append({
            "m8": m8, "xt8": xt8, "xr8": xr8, "xq16": xq16, "xq8": xq8c,
            "xtp": xtp, "xrp": xrp, "wvt": wvt, "patt8": patt.astype(NPFP8),
            "pattp": pattp.astype(NPBF16), "i8": i8, "i16": i16, "i32": i32,
        })
    return in_maps


def kernel(x, Wq, Wk, Wv, _trace=False, _trace_kwargs=None):
    if "nc" not in _cached:
        _cached["nc"] = _build_nc()
    nc = _cached["nc"]
    in_maps = _host_inputs(
        np.asarray(x, np.float32), np.asarray(Wq, np.float32),
        np.asarray(Wk, np.float32), np.asarray(Wv, np.float32),
    )
    kw = dict(_trace_kwargs or {})
    res = run_bass_kernel_spmd(
        nc, in_maps, core_ids=list(range(NCORES)), trace=_trace, **kw
    )
    out = np.empty((S, DE), np.float32)
    for c in range(NCORES):
        out[c::NCORES] = res.results[c]["out"].astype(np.float32)
    _cached["last_results"] = res
    return out


# revision 5
# speedup vs baseline: 1.0378x; 1.0378x over previous
"""Causal attention head (S=8192, De=dim=256) on 8 trn2 NeuronCores — fp8 version.

Math: score = (x Wq^T Wk x^T)/16, causal-masked softmax, out = attn @ (Wv x^T)^T.

Design (per core, rows c::8 stride-interleaved; kernel SPMD-identical, all
per-core variation in inputs):
  - Fold K: M8 = fp8(32*Wq^T@Wk) host-side; QmT8 = fp8(M8^T @ xq8^T) on device;
    st_psum = 512*score via ONE fp8 DoubleRow matmul per 128-col tile
    (contraction 256 = 2x128 groups packed in the free dim).
  - Fold V: B = P @ x accumulated in PSUM (DoubleRow fp8 on tile PAIRS);
    epilogue out-chunk = (pd*xq + B^T-read) @ Wv^T in bf16. Rowsums ride the
    B banks via pt-stationary matmuls with a 2-col ones moving operand.
  - Diagonal OUT of the fp8 fabric (pattern-zeroed): d = 512*s_qq via
    prod16 = qm_psum*xq16 + ones-matmul; pd = exp(d/512 - 1) on ACT; folded
    into rowsum (DVE add) and output (pd-scaled xq stationary into epi psum).
  - pt = exp(score - 1): off-diag scores in [-5.7, 5.9] land in fp8 normal
    range (max e^4.9=134 < 240, no overflow/underflow).
  - Causality: noncausal computed cells get -7680 (=-15 in score units) added
    pre-exp via one identity-stationary matmul per band tile (I8=32I x -240
    patterns); exp then gives 0 in fp8.
  - exp split across ACT (true exp, fp8 out) / DVE / GPSIMD (Schraudolph bits
    = st*a + b -> uint8, saturating at 0 on underflow; bitcast to fp8).
  - Rows with local u<64 (n_keys<=512) recomputed fully in bf16 (patch).
"""

import sys

sys.path.insert(0, "/opt/trn_rl_repo")

import math
from collections import deque
from contextlib import ExitStack

import ml_dtypes
import numpy as np

import concourse.bass as bass
import concourse.mybir as mybir
import concourse.tile as tile
from concourse import bacc
from concourse.bass_utils import run_bass_kernel_spmd

BF16 = mybir.dt.bfloat16
F32 = mybir.dt.float32
FP8 = mybir.dt.float8e4
U8 = mybir.dt.uint8
NPBF16 = ml_dtypes.bfloat16
NPFP8 = ml_dtypes.float8_e4m3
DR = mybir.MatmulPerfMode.DoubleRow
EXP = mybir.ActivationFunctionType.Exp
CPY = mybir.ActivationFunctionType.Copy
MUL = mybir.AluOpType.mult
ADD = mybir.AluOpType.add

S, DIM, DE = 8192, 256, 256
NCORES = 8
RPC = S // NCORES           # 1024 local rows per core
NRC = 4                     # row chunks of 256
G = 4                       # col tiles per exp group
MSCALE = 32.0               # M8 = fp8(32 * Wq^T Wk)  ->  st = 512*score
STS = 512.0
C_OFF = 1.0                 # global softmax shift: pt = exp(score - 1)
A_SCH = 8 * math.log2(math.e) / STS
B_SCH = 56.0 - 8 * math.log2(math.e) * C_OFF - 0.3
PATCH = 64
OFFS = [0, 16, 48, 96]      # even-group pattern offsets (widths 16,32,48,64)
OFFS_O = [160, 240, 336, 448]  # odd-group pattern offsets (widths 80,96,112,128)

_cached = {}


class Sched:
    """Greedy engine load balancer for elementwise work (load in us)."""

    def __init__(self, nc):
        self.nc = nc
        self.bias = None
        self.load = {"act": 0.0, "dve": 0.0, "pool": 0.0}
        self.rate = {"act": 0.833e-3, "dve": 1.0417e-3, "pool": 1.389e-3}
        self.ovh = {"act": 0.24, "dve": 0.20, "pool": 0.26}

    def pick(self, nelem, allowed=("act", "dve", "pool")):
        best, cost = None, None
        for e in allowed:
            c = self.load[e] + nelem * self.rate[e] + self.ovh[e]
            if cost is None or c < cost:
                best, cost = e, c
        self.load[best] = cost
        return best

    def add(self, eng, nelem):
        self.load[eng] += nelem * self.rate[eng] + self.ovh[eng]

    def exp(self, out_ap, in_ap, nelem):
        e = self.pick(nelem, ("act", "dve"))
        if e == "act":
            self.nc.scalar.activation(out_ap, in_ap, EXP, scale=1.0 / STS,
                                      bias=self.bias)
        elif e == "dve":
            self.nc.vector.tensor_scalar(
                out_ap.bitcast(U8), in_ap, A_SCH, B_SCH, MUL, ADD)
        else:
            self.nc.gpsimd.tensor_scalar(
                out_ap.bitcast(U8), in_ap, A_SCH, B_SCH, MUL, ADD)

    def copy(self, out_ap, in_ap, nelem, allowed=("act", "dve")):
        e = self.pick(nelem, allowed)
        if e == "act":
            self.nc.scalar.copy(out_ap, in_ap)
        elif e == "dve":
            self.nc.vector.tensor_copy(out_ap, in_ap)
        else:
            self.nc.gpsimd.tensor_copy(out_ap, in_ap)

    def mul2(self, out_ap, a_ap, b_ap, nelem):
        self.pick(nelem, ("dve",))
        self.nc.vector.tensor_mul(out_ap, a_ap, b_ap)


def _build_nc():
    nc = bacc.Bacc("TRN2", target_bir_lowering=False, debug=False,
                   num_devices=NCORES)
    # Packed DRAM inputs (fewer DMA instructions; HWDGE is serialized):
    #   cb8:  m8[512] | i8[128] | patt8[160] | xq8[2048]        fp8 [128, 2848]
    #   cb16a: xq16[2048] | wvt[512]                            bf16 [128, 2560]
    #   cbp:  xtp[1024] | xrp[1024] | pattp[160] | i16[128]     bf16 [128, 2336]
    cb8d = nc.dram_tensor("cb8", [128, 3264], FP8, kind="ExternalInput")
    cb16ad = nc.dram_tensor("cb16a", [128, 2560], BF16, kind="ExternalInput")
    cbpd = nc.dram_tensor("cbp", [128, 2848], BF16, kind="ExternalInput")
    i32d = nc.dram_tensor("i32", [128, 128], F32, kind="ExternalInput")
    xt8d = nc.dram_tensor("xt8", [2, 128, S], FP8, kind="ExternalInput")
    xr8d = nc.dram_tensor("xr8", [128, S * 2], FP8, kind="ExternalInput")
    outd = nc.dram_tensor("out", [RPC, DE], BF16, kind="ExternalOutput")

    with tile.TileContext(nc) as tc, ExitStack() as ctx:
        const = ctx.enter_context(tc.tile_pool(name="const", bufs=1))
        ps_st = ctx.enter_context(tc.tile_pool(name="ps_st", bufs=3, space="PSUM"))
        ps_b = ctx.enter_context(tc.tile_pool(name="ps_b", bufs=1, space="PSUM"))
        pt_pool = ctx.enter_context(tc.tile_pool(name="pt", bufs=7))
        sb_pool = ctx.enter_context(tc.tile_pool(name="sb", bufs=2))
        os_pool = ctx.enter_context(tc.tile_pool(name="os", bufs=3))
        _emit(nc, const, ps_st, ps_b, pt_pool, sb_pool,
              os_pool, cb8d, cb16ad, cbpd, i32d, xt8d, xr8d, outd)

    nc.compile()
    return nc


def _emit(nc, const, ps_st, ps_b, pt_pool, sb_pool,
          os_pool, cb8d, cb16ad, cbpd, i32d, xt8d, xr8d, outd):
    sch = Sched(nc)

    # ---- SBUF constants / staged inputs ----
    cb8 = const.tile([128, 3264], FP8, tag="cb8")
    cb16a = const.tile([128, 2560], BF16, tag="cb16a")
    cbp = const.tile([128, 2848], BF16, tag="cbp")
    m8 = cb8[:, 0:512]
    i8 = cb8[:, 512:640]
    patt8 = cb8[:, 640:1216]
    xq8 = cb8[:, 1216:3264]
    xq16 = cb16a[:, 0:2048]
    wvt = cb16a[:, 2048:2560]
    xtp = cbp[:, 0:1024]
    xrp = cbp[:, 1024:2048]
    pattp = cbp[:, 2048:2208]
    i16 = cbp[:, 2208:2336]
    m16 = cbp[:, 2336:2848]
    xt8 = const.tile([128, 2 * S], FP8, tag="xt8")
    xr8 = const.tile([128, 2 * S], FP8, tag="xr8")
    i32 = const.tile([128, 128], F32, tag="i32")
    qmt8 = const.tile([128, 2 * RPC], FP8, tag="qmt8")
    qmtp = const.tile([128, 2 * PATCH], BF16, tag="qmtp")
    prod16 = const.tile([128, 2 * RPC], BF16, tag="prod16")
    ones16 = const.tile([128, 1], BF16, tag="ones16")
    ones8 = const.tile([128, 2], FP8, tag="ones8")
    bias_m1 = const.tile([128, 1], F32, tag="bias_m1")
    pd_sb = const.tile([128, 2 * NRC], F32, tag="pd_sb")
    pdt_sb = [const.tile([1, 128], F32, tag=f"pdt{h}", name=f"pdt{h}")
              for h in range(2)]
    pdb_sb = [const.tile([128, 128], F32, tag=f"pdb{h}", name=f"pdb{h}")
              for h in range(2)]
    xqpd = const.tile([128, 256], BF16, tag="xqpd")
    rinv_sb = const.tile([128, 2], F32, tag="rinv_sb")
    rstot = const.tile([128, 2], F32, tag="rstot")

    nc.gpsimd.memset(ones16[:], 1.0)
    nc.gpsimd.memset(ones8[:], 1.0)
    nc.gpsimd.memset(bias_m1[:], -C_OFF)
    sch.bias = bias_m1[:]

    # ---- DMAs (single SP queue; HWDGE+DMA engines serialize globally,
    # so order = consumption order, minimal instruction count) ----
    nc.sync.dma_start(cb8[:], cb8d[:, :])
    for h in range(2):
        nc.sync.dma_start(xt8[:, h * S:h * S + 512], xt8d[h, :, 0:512])
    for h in range(2):
        nc.sync.dma_start(xt8[:, h * S + 512:h * S + 2048],
                          xt8d[h, :, 512:2048])
    nc.sync.dma_start(xr8[:, 0:4096], xr8d[:, 0:4096])
    nc.sync.dma_start(cb16a[:], cb16ad[:, :])
    nc.sync.dma_start(i32[:], i32d[:, :])
    for o in range(2048, S, 2048):
        for h in range(2):
            nc.sync.dma_start(xt8[:, h * S + o:h * S + o + 2048],
                              xt8d[h, :, o:o + 2048])
        nc.sync.dma_start(xr8[:, 2 * o:2 * o + 4096],
                          xr8d[:, 2 * o:2 * o + 4096])
        if o == 2048:
            nc.sync.dma_start(cbp[:], cbpd[:, :])

    # views
    xt8v = xt8[:].rearrange("p (h s) -> p h s", h=2)      # [128, 2, 8192]
    m8v = m8.rearrange("p (h e) -> p h e", h=2)           # [128, 2, 256]
    qmt8v = qmt8[:].rearrange("p (h q) -> p h q", h=2)    # [128, 2, 1024]
    xq8v = xq8.rearrange("p (h q) -> p h q", h=2)         # [128, 2, 1024]
    xr8v = xr8[:].rearrange("p (c e) -> p c e", e=256)    # [128, 64, 256]
    ones8v = ones8[:].rearrange("p (t o) -> p t o", t=2)  # [128, 2, 1]

    # PSUM banks: ps_st 3x2 + ps_b 2 = 8; qm/epi tiles rotate in ps_st
    b_ps = [ps_b.tile([128, 512], F32, tag=f"b{eh}", name=f"b{eh}")
            for eh in range(2)]

    def emit_qm(r):
        q0 = r * 256
        qm_ps = ps_st.tile([128, G * 256], F32, tag="st", name="qm")
        for eh in range(2):
            nc.tensor.matmul(
                qm_ps[:, eh * 256:(eh + 1) * 256],
                m8v[:, :, eh * 128:(eh + 1) * 128],
                xq8v[:, :, q0:q0 + 256],
                start=True, stop=True, perf_mode=DR)
        for eh in range(2):
            sch.copy(qmt8[:, eh * RPC + q0:eh * RPC + q0 + 256],
                     qm_ps[:, eh * 256:(eh + 1) * 256], 256)
        for eh in range(2):
            nc.gpsimd.tensor_mul(
                prod16[:, eh * RPC + q0:eh * RPC + q0 + 256],
                qmt8[:, eh * RPC + q0:eh * RPC + q0 + 256],
                xq8[:, eh * RPC + q0:eh * RPC + q0 + 256])
            sch.add("pool", 256)
        for h in range(2):
            for eh in range(2):
                nc.tensor.matmul(
                    b_ps[h][:, 257:258],
                    prod16[:, eh * RPC + q0 + h * 128:
                           eh * RPC + q0 + h * 128 + 128],
                    ones16[:, 0:1],
                    start=(r == 0 and eh == 0), stop=False,
                    skip_group_check=True)
        # pd chain: pd = exp(d/512 - 1); pdT via PE transpose; broadcast to
        # pdb via SWDGE (stride-0 source) on the otherwise-idle Pool engine.
        for h in range(2):
            nc.scalar.activation(pd_sb[:, 2 * r + h:2 * r + h + 1],
                                 b_ps[h][:, 257:258], EXP, scale=1.0 / STS,
                                 bias=bias_m1[:])
            sch.add("act", 4)
        for h in range(2):
            nc.tensor.matmul(
                qm_ps[0:1, 512 + h * 128:640 + h * 128],
                pd_sb[:, 2 * r + h:2 * r + h + 1], i32[:],
                start=True, stop=True, is_transpose=True,
                skip_group_check=True)
            nc.vector.tensor_copy(pdt_sb[h][:],
                                  qm_ps[0:1, 512 + h * 128:640 + h * 128])
            sch.add("dve", 64)
            nc.gpsimd.partition_broadcast(pdb_sb[h][:], pdt_sb[h][:])

    def emit_patch():
        # QmTp = (xq16 @ M16)^T in bf16 (clean of fp8 error) for patch rows
        qmp_ps = ps_st.tile([128, G * 256], F32, tag="st", name="qmp")
        for e2 in range(2):
            for i in range(2):
                nc.tensor.matmul(
                    qmp_ps[:, e2 * 64:(e2 + 1) * 64],
                    m16[:, i * 256 + e2 * 128:i * 256 + e2 * 128 + 128],
                    xq16[:, i * RPC:i * RPC + PATCH],
                    start=(i == 0), stop=(i == 1), skip_group_check=True)
        for e2 in range(2):
            nc.scalar.copy(qmtp[:, e2 * PATCH:(e2 + 1) * PATCH],
                           qmp_ps[:, e2 * 64:(e2 + 1) * 64])
            sch.add("act", PATCH)
        st_p = ps_st.tile([128, G * 256], F32, tag="st")
        for j in range(4):
            for eh in range(2):
                nc.tensor.matmul(
                    st_p[:, j * 64:(j + 1) * 64],
                    xtp[:, eh * 8 * PATCH + j * 128:
                        eh * 8 * PATCH + (j + 1) * 128],
                    qmtp[:, eh * PATCH:(eh + 1) * PATCH],
                    start=(eh == 0), stop=(eh == 1))
            wdt = 16 * (j + 1)
            nc.tensor.matmul(
                st_p[:, j * 64:j * 64 + wdt],
                i16, pattp[:, OFFS[j]:OFFS[j] + wdt],
                start=False, stop=False, skip_group_check=True)
        ptp = pt_pool.tile([128, G * 256], BF16, tag="ptp")
        nc.scalar.activation(ptp[:, 0:256], st_p[:, 0:256], EXP,
                             scale=1.0 / 16)
        sch.add("act", 256)
        for j in range(4):
            for eh in range(2):
                nc.tensor.matmul(
                    b_ps[eh][:, 0:PATCH],
                    xrp[:, j * 256 + eh * 128:j * 256 + eh * 128 + 128],
                    ptp[:, j * 64:(j + 1) * 64],
                    start=(j == 0), stop=(j == 3))
            nc.tensor.matmul(
                b_ps[0][0:PATCH, 256:257],
                ptp[:, j * 64:(j + 1) * 64], ones16[:, 0:1],
                start=False, stop=False, skip_group_check=True)
        bp_sb = sb_pool.tile([128, 512], BF16, tag="bsb")
        for eh in range(2):
            sch.copy(bp_sb[:, eh * PATCH:(eh + 1) * PATCH],
                     b_ps[eh][:, 0:PATCH], PATCH, allowed=("act", "dve"))
        epip = ps_st.tile([128, G * 256], F32, tag="st", name="epip")
        for eh in range(2):
            nc.tensor.matmul(
                epip[0:PATCH, 0:256],
                bp_sb[:, eh * PATCH:(eh + 1) * PATCH],
                wvt[:, eh * 256:(eh + 1) * 256],
                start=(eh == 0), stop=(eh == 1), skip_group_check=True)
        rp = rinv_sb[0:PATCH, 0:1]
        nc.vector.reciprocal(rp, b_ps[0][0:PATCH, 256:257])
        sch.add("dve", 8)
        osb = os_pool.tile([128, 256], BF16, tag="osb")
        nc.vector.tensor_scalar_mul(osb[0:PATCH, :], epip[0:PATCH, 0:256], rp)
        sch.add("dve", 256)
        nc.sync.dma_start(outd[0:PATCH, :], osb[0:PATCH, :])

    def emit_pv(pt, g, r, u0g, w, last):
        # pt tiles at 256-col slot stride; band slots offset by soff with a
        # zero pad below so rowsum lhsT slices stay full-128 / base-0
        soff = u0g % 128
        qbase = u0g - soff
        ptv = pt[:].rearrange("p (t x) -> p t x", x=256)[:, :, soff:soff + w]
        ptz = pt[:].rearrange("p (t x) -> p t x", x=256)
        for p in range(2):
            j0 = G * g + 2 * p
            for eh in range(2):
                nc.tensor.matmul(
                    b_ps[eh][:, u0g:256],
                    xr8v[:, j0:j0 + 2, eh * 128:(eh + 1) * 128],
                    ptv[:, 2 * p:2 * p + 2, :],
                    start=(g == 0 and p == 0), stop=(last and p == 1),
                    perf_mode=DR, skip_group_check=True)
            for h in range(2):
                if (h + 1) * 128 <= qbase:
                    continue
                c0 = h * 128 - qbase
                nc.tensor.matmul(
                    b_ps[h][:, 256:257],
                    ptz[:, 2 * p:2 * p + 2, c0:c0 + 128],
                    ones8v,
                    start=False, stop=False, perf_mode=DR,
                    skip_group_check=True)

    def emit_chunk(r):
        ngroups = 4 * (r + 1)
        pending = deque()
        for g in range(ngroups):
            band_pos = g - 4 * r
            u0g = 64 * band_pos if band_pos >= 0 else 0
            w = 256 - u0g
            soff = u0g % 128          # pt slot offset; zeros below for rowsums
            st = ps_st.tile([128, G * 256], F32, tag="st")
            for t in range(G):
                j = G * g + t
                nc.tensor.matmul(
                    st[:, t * 256:t * 256 + w],
                    xt8v[:, :, j * 128:(j + 1) * 128],
                    qmt8v[:, :, r * 256 + u0g:(r + 1) * 256],
                    start=True, stop=True, perf_mode=DR)
                if band_pos >= 0:
                    wdt = 16 * (t + 1)
                    nc.tensor.matmul(
                        st[:, t * 256:t * 256 + wdt],
                        i8, patt8[:, OFFS[t]:OFFS[t] + wdt],
                        start=False, stop=False, skip_group_check=True)
            if g == 1 and r < NRC - 1:
                emit_qm(r + 1)
            if len(pending) >= 5:
                emit_pv(*pending.popleft())
            pt = pt_pool.tile([128, G * 256], FP8, tag="pt")
            if soff:
                nc.gpsimd.memset(
                    pt[:].rearrange("p (t x) -> p t x", x=256)[:, :, 0:soff],
                    0.0)
                sch.add("pool", 4 * soff // 4)
            stv = st[:].rearrange("p (t x) -> p t x", x=256)[:, :, 0:w]
            ptv = pt[:].rearrange("p (t x) -> p t x", x=256)[:, :, soff:soff + w]
            sch.exp(ptv, stv, 4 * w)
            pending.append((pt, g, r, u0g, w, g == ngroups - 1))
        while pending:
            emit_pv(*pending.popleft())

    def emit_epilogue(r):
        epi = ps_st.tile([128, G * 256], F32, tag="st", name="epi")
        for h in range(2):
            nc.vector.tensor_add(
                rstot[:, h:h + 1], b_ps[h][:, 256:257],
                pd_sb[:, 2 * r + h:2 * r + h + 1])
            nc.vector.reciprocal(rinv_sb[:, h:h + 1], rstot[:, h:h + 1])
            sch.add("dve", 16)
            for eh in range(2):
                nc.gpsimd.tensor_mul(
                    xqpd[:, eh * 128:(eh + 1) * 128],
                    xq16[:, eh * RPC + r * 256 + h * 128:
                         eh * RPC + r * 256 + h * 128 + 128],
                    pdb_sb[h][:])
                sch.add("pool", 128)
            b_sb = sb_pool.tile([128, 512], BF16, tag="bsb")
            for eh in range(2):
                sch.copy(b_sb[:, eh * 256:(eh + 1) * 256],
                         b_ps[eh][:, 0:256], 256, allowed=("dve",))
            er = epi[:, h * 256:(h + 1) * 256]
            for eh in range(2):
                nc.tensor.matmul(
                    er, xqpd[:, eh * 128:(eh + 1) * 128],
                    wvt[:, eh * 256:(eh + 1) * 256],
                    start=(eh == 0), stop=False, skip_group_check=True)
            for eh in range(2):
                nc.tensor.matmul(
                    er, b_sb[:, eh * 256 + h * 128:eh * 256 + h * 128 + 128],
                    wvt[:, eh * 256:(eh + 1) * 256],
                    start=False, stop=(eh == 1), skip_group_check=True)
            osb = os_pool.tile([128, 256], BF16, tag="osb")
            rows0 = r * 256 + h * 128
            if r == 0 and h == 0:
                nc.vector.tensor_scalar_mul(
                    osb[PATCH:128, :], er[PATCH:128, :],
                    rinv_sb[PATCH:128, h:h + 1])
                sch.add("dve", 256)
                nc.sync.dma_start(outd[rows0 + PATCH:rows0 + 128, :],
                                  osb[PATCH:128, :])
            else:
                sch.pick(256, ("dve",))
                nc.vector.tensor_scalar_mul(osb[:], er,
                                            rinv_sb[:, h:h + 1])
                nc.sync.dma_start(outd[rows0:rows0 + 128, :], osb[:])

    # ---------- emission order: qm(r+1) emitted mid-chunk(r); patch after
    # epilogue(1) so its consts (cbp DMA) have arrived ----------
    emit_qm(0)
    for r in range(NRC):
        emit_chunk(r)
        emit_epilogue(r)
        if r == 1:
            emit_patch()


def _host_inputs(x, Wq, Wk, Wv):
    def two_half(a, width):            # [256, w] -> [128, 2*w] (half-major)
        return np.ascontiguousarray(
            a.reshape(2, 128, width).transpose(1, 0, 2).reshape(128, 2 * width))

    xT = np.ascontiguousarray(x.T)
    m8 = two_half((MSCALE * (Wq.T @ Wk)).astype(np.float32), 256)
    i8 = 32.0 * np.eye(128, dtype=np.float32)
    i16 = 16.0 * np.eye(128, dtype=np.float32)
    i32 = np.eye(128, dtype=np.float32)
    wvt = two_half(np.ascontiguousarray(Wv.T), 256)
    xtp = two_half(xT[:, :8 * PATCH], 8 * PATCH)
    xrp = np.ascontiguousarray(
        x[:4 * 128].reshape(4, 128, 256).transpose(1, 0, 2).reshape(128, 1024))
    xt8 = np.ascontiguousarray(xT.astype(NPFP8).reshape(2, 128, S))
    xr8 = np.ascontiguousarray(
        x.reshape(64, 128, 256).transpose(1, 0, 2).reshape(128, 2 * S)
    ).astype(NPFP8)
    k_idx = np.arange(128)[:, None]
    in_maps = []
    for c in range(NCORES):
        xq = np.ascontiguousarray(x[c::NCORES].T)    # [256, 1024]
        xq16 = two_half(xq, RPC)
        patt = np.zeros((128, 576), np.float32)      # main: kill k >= 8w+c
        pattp = np.zeros((128, 160), np.float32)     # patch: kill k > 8w+c
        for t in range(4):
            wdt = 16 * (t + 1)
            w_idx = np.arange(wdt)[None, :]
            kk = 128 * t + k_idx
            patt[:, OFFS[t]:OFFS[t] + wdt] = np.where(
                kk >= (8 * w_idx + c), -240.0, 0.0)
            pattp[:, OFFS[t]:OFFS[t] + wdt] = np.where(
                kk > (8 * w_idx + c), -240.0, 0.0)
            # odd band groups start 64 rows before the diagonal block:
            # kill iff 128t + 512 + k >= 8w + c, width 16(t+1)+64
            wdt_o = 16 * (t + 1) + 64
            w_idx = np.arange(wdt_o)[None, :]
            patt[:, OFFS_O[t]:OFFS_O[t] + wdt_o] = np.where(
                (128 * t + 512 + k_idx) >= (8 * w_idx + c), -240.0, 0.0)
        cb8 = np.concatenate(
            [m8, i8, patt, xq16], axis=1).astype(NPFP8)
        cb16a = np.concatenate([xq16, wvt], axis=1).astype(NPBF16)
        m16 = two_half((Wq.T @ Wk).astype(np.float32), 256)
        cbp = np.concatenate([xtp, xrp, pattp, i16, m16],
                             axis=1).astype(NPBF16)
        in_maps.append({
            "cb8": np.ascontiguousarray(cb8),
            "cb16a": np.ascontiguousarray(cb16a),
            "cbp": np.ascontiguousarray(cbp),
            "i32": i32, "xt8": xt8, "xr8": xr8,
        })
    return in_maps


def _host_inputs_old(x, Wq, Wk, Wv):
    M8 = (MSCALE * (Wq.T @ Wk)).astype(NPFP8)
    m8 = np.ascontiguousarray(
        M8.reshape(2, 128, 256).transpose(1, 0, 2).reshape(128, 512))
    xT = np.ascontiguousarray(x.T)
    xt8 = np.ascontiguousarray(xT.astype(NPFP8).reshape(2, 128, S))
    xr8 = np.ascontiguousarray(
        x.reshape(64, 128, 256).transpose(1, 0, 2).reshape(128, 2 * S)
    ).astype(NPFP8)
    xtp = np.ascontiguousarray(
        xT[:, :8 * PATCH].astype(NPBF16).reshape(2, 128, 8 * PATCH))
    xrp = np.ascontiguousarray(
        x[:4 * 128].reshape(4, 128, 256).transpose(1, 0, 2).reshape(128, 1024)
    ).astype(NPBF16)
    wvt = np.ascontiguousarray(Wv.T).astype(NPBF16).reshape(2, 128, 256)
    i8 = (32.0 * np.eye(128, dtype=np.float32)).astype(NPFP8)
    i16 = (16.0 * np.eye(128, dtype=np.float32)).astype(NPBF16)
    i32 = np.eye(128, dtype=np.float32)
    k_idx = np.arange(128)[:, None]
    in_maps = []
    for c in range(NCORES):
        xq = np.ascontiguousarray(x[c::NCORES].T)
        xq16 = xq.astype(NPBF16).reshape(2, 128, RPC)
        xq8c = xq.astype(NPFP8).reshape(2, 128, RPC)
        patt = np.zeros((128, 160), np.float32)    # main: kill k >= 8w+c (diag out)
        pattp = np.zeros((128, 160), np.float32)   # patch: kill k > 8w+c (diag in)
        for t in range(4):
            wdt = 16 * (t + 1)
            w_idx = np.arange(wdt)[None, :]
            kk = 128 * t + k_idx
            patt[:, OFFS[t]:OFFS[t] + wdt] = np.where(
                kk >= (8 * w_idx + c), -240.0, 0.0)
            pattp[:, OFFS[t]:OFFS[t] + wdt] = np.where(
                kk > (8 * w_idx + c), -240.0, 0.0)
        in_maps.append({
            "m8": m8, "xt8": xt8, "xr8": xr8, "xq16": xq16, "xq8": xq8c,
            "xtp": xtp, "xrp": xrp, "wvt": wvt, "patt8": patt.astype(NPFP8),
            "pattp": pattp.astype(NPBF16), "i8": i8, "i16": i16, "i32": i32,
        })
    return in_maps


def kernel(x, Wq, Wk, Wv, _trace=False, _trace_kwargs=None):
    if "nc" not in _cached:
        _cached["nc"] = _build_nc()
    nc = _cached["nc"]
    in_maps = _host_inputs(
        np.asarray(x, np.float32), np.asarray(Wq, np.float32),
        np.asarray(Wk, np.float32), np.asarray(Wv, np.float32),
    )
    kw = dict(_trace_kwargs or {})
    res = run_bass_kernel_spmd(
        nc, in_maps, core_ids=list(range(NCORES)), trace=_trace, **kw
    )
    out = np.empty((S, DE), np.float32)
    for c in range(NCORES):
        out[c::NCORES] = res.results[c]["out"].astype(np.float32)
    _cached["last_results"] = res
    return out
